# revision 15
# baseline (speedup 1.0000x reference)
"""Trainium2 Bass kernel: dense multi-head dot-product attention.

Problem: x [4, 2048, 1024], W_Q/W_K/W_V [16, 1024, 64] ->
         out [4, 2048, 1024] (heads concatenated on the feature dim).

Sharding: 8 cores = 4 batches x 2 head-groups (8 heads each).
Per core, everything is computed in "transposed" layouts so that no
on-chip transpose of the big attention matrix is ever needed:
  - host passes x^T [1024, 2048] (n on partitions) per batch
  - Q^T/K^T/V^T [2*64, 2048] per head pair (W stationary, x^T moving)
  - Vaug[k, kc, hp*65+d] built from V^T via PE transposes; col hp*65+64
    holds ones so row 64 of the PV accumulator is the softmax denom.
  - scores S^T[k, m] = sum_d K^T[d,k] Q^T[d,m]  (k on partitions).
    The two heads of a pair use PE row groups {0,1} / {2,3} so their
    score matmuls run CONCURRENTLY in the systolic array (row tiling).
  - P^T = exp(S^T/8): one ScalarE activation per (kc, mq) covering
    both heads. ScalarE streams ~33.5M exps per core; the score/exp
    pipeline is a single flat 256-step stream across all mq blocks and
    head pairs so ScalarE never pauses at block boundaries.
    Softmax skips max-subtraction: |S/8| < ~12, exp safe in fp32,
    softmax shift-invariant.
  - O^T accumulated in PSUM over the 16 key chunks per head; PV lags
    the score stream by LOOKAHEAD steps.
  - normalize: recip (DVE) -> partition_broadcast (GpSimd) -> mul
    (DVE, numerator direct from PSUM) -> DMA out. Host transposes
    when gathering.

Schedule notes (v3):
  - Input DMA: few descriptors (xt is ONE [128,8,2048] SBUF tile in 5
    descriptors, 12 weight descriptors), issued on sync + gpsimd
    queues before any other emission; ordered so K/Q block-0
    projections start as soon as ~1.25MB lands.
  - Upfront only: K block0 + Q block0 projections -> score pipeline
    starts; V block0 + transposes emitted behind the first scores;
    the rest of pair-0's projection work drains through an urgent
    queue (K blocks first, then V+transposes, then Q) at 2 items per
    step; later pairs' projections spread across the previous pair's
    64 steps as fine-grained items (<= 4 matmuls each).
  - Drains interleave the two heads' recip/broadcast chains so the
    GpSimd broadcast latency hides under DVE work.

Matmul operands are bf16 (PSUM accumulation fp32): measured ~1.1e-2
rel err vs the 2e-2 gate. fp8 was tested numerically and fails the
gate (concentrated softmax rows copy single V rows, so fp8 V/P
quantization error does not average out). fp32/f32r matmuls lower to
half rate, so bf16 is the fast path.

PSUM budget (8 banks): st ring 2 x [128,2,512] fp32 (2 banks each)
+ proj/transpose accum "ps" (1 bank) + 3 x ot [65,512] (1 bank each).
"""

import os
from contextlib import ExitStack

import numpy as np

import concourse.bass as bass  # noqa: F401  (bass types via bacc)
import concourse.tile as tile
from concourse import bacc, mybir
from concourse import bass_utils
from concourse.masks import make_identity

F32 = mybir.dt.float32
BF16 = mybir.dt.bfloat16

B, M, N, H, D = 4, 2048, 1024, 16, 64
NCORES = 8
NCH = 8          # d_model / 128 chunks
KC = 16          # key chunks of 128
MQ = 4           # m blocks of 512
NPAIR = 4
STEPS = NPAIR * MQ * KC  # 256 global score/exp steps
LOOKAHEAD = 2
SCALE = 0.125    # 1/sqrt(64)
EXPF = mybir.ActivationFunctionType.Exp


def build_nc():
    nc = bacc.Bacc(
        "TRN2", target_bir_lowering=False, debug=False, enable_asserts=False
    )
    xt_d = nc.dram_tensor("xt", [N, M], BF16, kind="ExternalInput")
    wq_d = nc.dram_tensor("wq", [4, N, 128], BF16, kind="ExternalInput")
    wk_d = nc.dram_tensor("wk", [4, N, 128], BF16, kind="ExternalInput")
    wv_d = nc.dram_tensor("wv", [4, N, 128], BF16, kind="ExternalInput")
    o_d = nc.dram_tensor("ot", [8 * D, M], F32, kind="ExternalOutput")

    with tile.TileContext(nc) as tc, ExitStack() as ctx:
        const_pool = ctx.enter_context(tc.tile_pool(name="constp", bufs=1))
        xt_pool = ctx.enter_context(tc.tile_pool(name="xtp", bufs=1))
        w_pool = ctx.enter_context(tc.tile_pool(name="wp", bufs=12))
        qkv_pool = ctx.enter_context(tc.tile_pool(name="qkvp", bufs=2))
        vaug_pool = ctx.enter_context(tc.tile_pool(name="vaugp", bufs=2))
        pt_pool = ctx.enter_context(tc.tile_pool(name="ptp", bufs=5))
        out_pool = ctx.enter_context(tc.tile_pool(name="outp", bufs=8))
        small_pool = ctx.enter_context(tc.tile_pool(name="smallp", bufs=8))
        st_pool = ctx.enter_context(tc.tile_pool(name="stp", bufs=2, space="PSUM"))
        ot_pool = ctx.enter_context(tc.tile_pool(name="otp", bufs=3, space="PSUM"))

        # ---- batched input DMA first (before any const setup) so the
        # descriptor issue overlaps the framework preamble. xt quarter
        # 0 is split by chunk-halves so the first K matmuls can start
        # after ~0.75MB lands.
        xt_sb = xt_pool.tile([128, NCH, M], BF16, name="xt_sb")

        def load_xq(eng, g, clo=0, chi=NCH):
            eng.dma_start(
                xt_sb[:, clo:chi, g * 512:(g + 1) * 512],
                xt_d.ap()[clo * 128:chi * 128, g * 512:(g + 1) * 512].rearrange(
                    "(c p) m -> p c m", p=128
                ),
            )

        wts = {}

        def load_w(nm, p):
            wd = {"q": wq_d, "k": wk_d, "v": wv_d}[nm]
            wt = w_pool.tile([128, NCH, 128], BF16, name=f"w{nm}{p}", tag="wt")
            nc.gpsimd.dma_start(
                wt[:], wd.ap()[p].rearrange("(c p) d -> p c d", p=128)
            )
            wts[(nm, p)] = wt

        load_xq(nc.sync, 0, 0, 4)
        load_w("k", 0)
        load_xq(nc.sync, 0, 4, 8)
        load_w("q", 0)
        load_w("v", 0)
        load_xq(nc.sync, 1)
        load_xq(nc.sync, 2)
        load_xq(nc.sync, 3)

        ident = const_pool.tile([128, 128], BF16, name="ident")
        make_identity(nc, ident[:])
        ones16 = const_pool.tile([128, KC, 1], F32, name="ones16")
        nc.gpsimd.memset(ones16[:], 1.0)

        for p in range(1, 4):
            for nm in ("k", "v", "q"):
                load_w(nm, p)

        # ---------------------------------------------------------------
        # Projection work as fine-grained emission items. Chain-split
        # qkv projections keep the tag="ps" PSUM accumulator between
        # their two items, so items of one chain stay adjacent.
        # ---------------------------------------------------------------
        def make_pair_tiles(p):
            qt = qkv_pool.tile([128, M], BF16, name="qt", tag="qt")
            kt = qkv_pool.tile([128, M], BF16, name="kt", tag="kt")
            vt = qkv_pool.tile([128, M], BF16, name="vt", tag="vt")
            vaug = vaug_pool.tile([128, KC, 130], BF16, name="vaug", tag="vaug")
            return qt, kt, vt, vaug

        def qkv_items(p, g, nm, dst):
            """Two items: contraction chunks 0-3, then 4-7 + copy out."""
            psbox = {}

            def half_a():
                wt = wts[(nm, p)]
                ps = st_pool.tile(
                    [128, 512], F32, name="ps_prj", tag="ps", bufs=1
                )
                psbox[0] = ps
                for c in range(4):
                    nc.tensor.matmul(
                        ps[:],
                        lhsT=wt[:, c, :],
                        rhs=xt_sb[:, c, g * 512:(g + 1) * 512],
                        start=(c == 0),
                        stop=False,
                        skip_group_check=True,
                    )

            def half_b():
                wt = wts[(nm, p)]
                ps = psbox[0]
                for c in range(4, NCH):
                    nc.tensor.matmul(
                        ps[:],
                        lhsT=wt[:, c, :],
                        rhs=xt_sb[:, c, g * 512:(g + 1) * 512],
                        start=False,
                        stop=(c == NCH - 1),
                        skip_group_check=True,
                    )
                nc.vector.tensor_copy(dst[:, g * 512:(g + 1) * 512], ps[:])

            return [half_a, half_b]

        def tr_items(g, vt, vaug):
            """Two items of two PE transposes each. Each transpose gets
            a fresh tag="ps" tile: matmul start=True zeroes the whole
            2KB PSUM zero-region, so transposes cannot share a bank."""

            def tr_half(k4r):
                for k4 in k4r:
                    kc = g * 4 + k4
                    trp = st_pool.tile(
                        [128, 128], BF16, name="trp", tag="ps", bufs=1
                    )
                    nc.tensor.transpose(
                        trp[:], vt[:, kc * 128:(kc + 1) * 128], ident[:]
                    )
                    nc.vector.tensor_copy(
                        vaug[:, kc, :].rearrange(
                            "p (h x) -> p h x", h=2
                        )[:, :, 0:64],
                        trp.rearrange("p (h d) -> p h d", h=2),
                    )

            return [lambda: tr_half((0, 1)), lambda: tr_half((2, 3))]

        def ones_item(vaug):
            def it():
                for hp in range(2):
                    nc.vector.tensor_copy(
                        vaug[:, :, hp * 65 + 64:hp * 65 + 65], ones16[:]
                    )
            return it

        def proj_items(p, qt, kt, vt, vaug):
            """Full projection item list for pairs >= 1."""
            items = [ones_item(vaug)]
            for g in range(4):
                items += qkv_items(p, g, "k", kt)
                items += qkv_items(p, g, "v", vt)
                items += tr_items(g, vt, vaug)
                items += qkv_items(p, g, "q", qt)
            return items

        # Drain of a finished [65,512] PV accumulator pair. Deferred one
        # mq (gated on the next mq's first exp output) so it can never
        # race the PV-stop matmul's systolic drain into PSUM.
        _PENDING_DRAIN = [None]

        def emit_drain(ots, p, mq, gate_pt=None):
            msl = slice(mq * 512, (mq + 1) * 512)
            if gate_pt is not None:
                gate = small_pool.tile([1, 8], F32, name="gate", tag="gate")
                nc.vector.tensor_copy(gate[:], gate_pt[0:1, 0, 0:8])
            rbcs = []
            for hp in range(2):
                sumsb = small_pool.tile([1, 512], F32, name="sumsb", tag="sm")
                nc.vector.tensor_copy(sumsb[:], ots[hp][64:65, :])
                recipb = small_pool.tile([1, 512], F32, name="recipb", tag="sm")
                scratch = small_pool.tile([1, 512], F32, name="scr", tag="sm")
                nc.vector.reciprocal_approx_accurate(
                    recipb[:], sumsb[:], scratch[:]
                )
                rbc = out_pool.tile([64, 512], F32, name="rbc", tag="o64")
                nc.gpsimd.partition_broadcast(rbc[:], recipb[:])
                rbcs.append(rbc)
            for hp in range(2):
                h = 2 * p + hp
                stage = out_pool.tile([64, 512], F32, name="stage", tag="o64")
                nc.vector.tensor_mul(stage[:], ots[hp][0:64, :], rbcs[hp])
                nc.sync.dma_start(o_d.ap()[h * 64:(h + 1) * 64, msl], stage[:])

        # ---- pair 0 upfront: K block0 + Q block0 (gates the score
        # pipeline); V block0 + transposes follow the first scores.
        tiles = {0: make_pair_tiles(0)}
        qt0, kt0, vt0, vaug0 = tiles[0]
        ones_item(vaug0)()
        for itm in qkv_items(0, 0, "k", kt0) + qkv_items(0, 0, "q", qt0):
            itm()
        vb0_items = qkv_items(0, 0, "v", vt0) + tr_items(0, vt0, vaug0)
        # Urgent ordering constraint: items are popped 2 per step, and
        # emission order IS execution order per engine queue, so an
        # item must be EMITTED before the step that emits its first
        # consumer. tr(g) is consumed by PV(kc=4g) at step 4g; with the
        # interleaved order below tr(g) lands at slot 4-3g+... =
        # {g1: slot 2, g2: 5, g3: 8} < {4, 8, 12}. K(g) is consumed by
        # scores(4g) emitted at step 4g-3: K lands at {0, 3, 6}. Q(g)
        # is consumed by scores(16g) emitted at step 16g-3: Q lands at
        # {9, 10, 11} < 13.
        urgent = []
        for g in range(1, 4):
            urgent += qkv_items(0, g, "k", kt0)
            urgent += qkv_items(0, g, "v", vt0)
            urgent += tr_items(g, vt0, vaug0)
        for g in range(1, 4):
            urgent += qkv_items(0, g, "q", qt0)

        # ---- flat score/exp pipeline over all 256 (pair, mq, kc) steps
        pts = {}

        def score_step(j):
            p, r = divmod(j, MQ * KC)
            mq, kc = divmod(r, KC)
            qt, kt = tiles[p][0], tiles[p][1]
            msl = slice(mq * 512, (mq + 1) * 512)
            ksl = slice(kc * 128, (kc + 1) * 128)
            st = st_pool.tile([128, 2, 512], F32, name="st", tag="st")
            for hp in range(2):
                hsl = slice(64 * hp, 64 * (hp + 1))
                nc.tensor.matmul(
                    st[:, hp, :],
                    lhsT=kt[hsl, ksl],
                    rhs=qt[hsl, msl],
                    start=True,
                    stop=True,
                )
            pt = pt_pool.tile([128, 2, 512], BF16, name="pt", tag="pt")
            nc.scalar.activation(pt[:], st[:], EXPF, scale=SCALE)
            pts[j] = pt

        for j in range(LOOKAHEAD):
            score_step(j)
        for itm in vb0_items:
            itm()

        pending, slots = [], {}
        ots = None
        for i in range(STEPS):
            p, r = divmod(i, MQ * KC)
            mq, kc = divmod(r, KC)
            if r == 0 and p + 1 < NPAIR:
                # allocate next pair's tiles and spread its projection
                # items over this pair's steps (finishing ~4 steps
                # early so the cross-pair score lookahead has data).
                tiles[p + 1] = make_pair_tiles(p + 1)
                pending = proj_items(p + 1, *tiles[p + 1])
                first = (len(urgent) + 1) // 2 if urgent else 0
                span = MQ * KC - first - 5
                slots = {
                    i + first + int(round((t + 1) * span / len(pending))): t
                    for t in range(len(pending))
                }
            if kc == 0:
                ot0 = ot_pool.tile([65, 512], F32, name="ot0", tag="ot")
                ot1 = ot_pool.tile([65, 512], F32, name="ot1", tag="ot")
                ots = (ot0, ot1)
            pt = pts.pop(i)
            if kc == 0 and _PENDING_DRAIN[0] is not None:
                emit_drain(*_PENDING_DRAIN[0], gate_pt=pt)
                _PENDING_DRAIN[0] = None
            vaug = tiles[p][3]
            for hp in range(2):
                nc.tensor.matmul(
                    ots[hp][:],
                    lhsT=vaug[:, kc, hp * 65:hp * 65 + 65],
                    rhs=pt[:, hp, :],
                    start=(kc == 0),
                    stop=(kc == KC - 1),
                    skip_group_check=True,
                )
            if i + LOOKAHEAD < STEPS:
                score_step(i + LOOKAHEAD)
            # 2 urgent (pair-0 tail) items per step, then spread items
            if urgent:
                urgent.pop(0)()
            if urgent:
                urgent.pop(0)()
            if i in slots:
                pending[slots[i]]()
            if kc == KC - 1:
                _PENDING_DRAIN[0] = (ots, p, mq)
        # flush the last mq's drain (no later pt to gate on)
        if _PENDING_DRAIN[0] is not None:
            emit_drain(*_PENDING_DRAIN[0], gate_pt=None)
            _PENDING_DRAIN[0] = None
    nc.compile()
    return nc


_NC_CACHE = None


def _get_nc():
    global _NC_CACHE
    if _NC_CACHE is None:
        _NC_CACHE = build_nc()
    return _NC_CACHE


def make_in_maps(x, W_Q, W_K, W_V):
    import ml_dtypes

    BF = ml_dtypes.bfloat16
    x = np.asarray(x, dtype=np.float32)
    W_Q = np.asarray(W_Q, dtype=np.float32)
    W_K = np.asarray(W_K, dtype=np.float32)
    W_V = np.asarray(W_V, dtype=np.float32)

    def prep_w(W, g):
        blk = W[8 * g:8 * g + 8]  # [8, 1024, 64]
        # pair-major [4, 1024, 128]: col = (head%2)*64 + d
        return np.ascontiguousarray(
            blk.reshape(4, 2, N, D).transpose(0, 2, 1, 3).reshape(4, N, 2 * D)
        ).astype(BF)

    xts = [np.ascontiguousarray(x[b].T).astype(BF) for b in range(B)]
    ws = [
        (prep_w(W_Q, g), prep_w(W_K, g), prep_w(W_V, g)) for g in range(2)
    ]
    in_maps = []
    for c in range(NCORES):
        b, g = divmod(c, 2)
        in_maps.append(
            {
                "xt": xts[b],
                "wq": ws[g][0],
                "wk": ws[g][1],
                "wv": ws[g][2],
            }
        )
    return in_maps


def gather_out(results):
    out = np.empty((B, M, N), dtype=np.float32)
    for c in range(NCORES):
        b, g = divmod(c, 2)
        out[b, :, 512 * g:512 * (g + 1)] = results[c]["ot"].T
    return out


def run(x, W_Q, W_K, W_V, **spmd_kwargs):
    nc = _get_nc()
    in_maps = make_in_maps(x, W_Q, W_K, W_V)
    res = bass_utils.run_bass_kernel_spmd(
        nc, in_maps, core_ids=list(range(NCORES)), **spmd_kwargs
    )
    return gather_out(res.results), res


def kernel(x, W_Q, W_K, W_V):
    out, _ = run(x, W_Q, W_K, W_V)
    return out


# revision 17
# speedup vs baseline: 1.0371x; 1.0371x over previous
"""Trainium2 Bass kernel: dense multi-head dot-product attention.

Problem: x [4, 2048, 1024], W_Q/W_K/W_V [16, 1024, 64] ->
         out [4, 2048, 1024] (heads concatenated on the feature dim).

Sharding: 8 cores = 4 batches x 2 head-groups (8 heads each).
Per core, everything is computed in "transposed" layouts so that no
on-chip transpose of the big attention matrix is ever needed:
  - host passes x^T [1024, 2048] (n on partitions) per batch
  - Q^T/K^T/V^T [2*64, 2048] per head pair (W stationary, x^T moving)
  - Vaug[k, kc, hp*65+d] built from V^T via PE transposes; col hp*65+64
    holds ones so row 64 of the PV accumulator is the softmax denom.
  - scores S^T[k, m] = sum_d K^T[d,k] Q^T[d,m]  (k on partitions).
    The two heads of a pair use PE row groups {0,1} / {2,3} so their
    score matmuls run CONCURRENTLY in the systolic array (row tiling).
  - P^T = exp(S^T/8): one ScalarE activation per (kc, mq) covering
    both heads. ScalarE streams ~33.5M exps per core; the score/exp
    pipeline is a single flat 256-step stream across all mq blocks and
    head pairs so ScalarE never pauses at block boundaries.
    Softmax skips max-subtraction: |S/8| < ~12, exp safe in fp32,
    softmax shift-invariant.
  - O^T accumulated in PSUM over the 16 key chunks per head; PV lags
    the score stream by LOOKAHEAD steps.
  - normalize: recip (DVE) -> partition_broadcast (GpSimd) -> mul
    (DVE, numerator direct from PSUM) -> DMA out. Host transposes
    when gathering.

Schedule notes (v3):
  - Input DMA: few descriptors (xt is ONE [128,8,2048] SBUF tile in 5
    descriptors, 12 weight descriptors), issued on sync + gpsimd
    queues before any other emission; ordered so K/Q block-0
    projections start as soon as ~1.25MB lands.
  - Upfront only: K block0 + Q block0 projections -> score pipeline
    starts; V block0 + transposes emitted behind the first scores;
    the rest of pair-0's projection work drains through an urgent
    queue (K blocks first, then V+transposes, then Q) at 2 items per
    step; later pairs' projections spread across the previous pair's
    64 steps as fine-grained items (<= 4 matmuls each).
  - Drains interleave the two heads' recip/broadcast chains so the
    GpSimd broadcast latency hides under DVE work.

Matmul operands are bf16 (PSUM accumulation fp32): measured ~1.1e-2
rel err vs the 2e-2 gate. fp8 was tested numerically and fails the
gate (concentrated softmax rows copy single V rows, so fp8 V/P
quantization error does not average out). fp32/f32r matmuls lower to
half rate, so bf16 is the fast path.

PSUM budget (8 banks): st ring 2 x [128,2,512] fp32 (2 banks each)
+ proj/transpose accum "ps" (1 bank) + 3 x ot [65,512] (1 bank each).
"""

import os
from contextlib import ExitStack

import numpy as np

import concourse.bass as bass  # noqa: F401  (bass types via bacc)
import concourse.tile as tile
from concourse import bacc, mybir
from concourse import bass_utils
from concourse.masks import make_identity

F32 = mybir.dt.float32
BF16 = mybir.dt.bfloat16

B, M, N, H, D = 4, 2048, 1024, 16, 64
NCORES = 8
NCH = 8          # d_model / 128 chunks
KC = 16          # key chunks of 128
MQ = 4           # m blocks of 512
NPAIR = 4
STEPS = NPAIR * MQ * KC  # 256 global score/exp steps
LOOKAHEAD = 3
SCALE = 0.125    # 1/sqrt(64)
EXPF = mybir.ActivationFunctionType.Exp


def build_nc():
    nc = bacc.Bacc(
        "TRN2", target_bir_lowering=False, debug=False, enable_asserts=False
    )
    xt_d = nc.dram_tensor("xt", [N, M], BF16, kind="ExternalInput")
    wq_d = nc.dram_tensor("wq", [4, N, 128], BF16, kind="ExternalInput")
    wk_d = nc.dram_tensor("wk", [4, N, 128], BF16, kind="ExternalInput")
    wv_d = nc.dram_tensor("wv", [4, N, 128], BF16, kind="ExternalInput")
    o_d = nc.dram_tensor("ot", [8 * D, M], F32, kind="ExternalOutput")

    with tile.TileContext(nc) as tc, ExitStack() as ctx:
        const_pool = ctx.enter_context(tc.tile_pool(name="constp", bufs=1))
        xt_pool = ctx.enter_context(tc.tile_pool(name="xtp", bufs=1))
        w_pool = ctx.enter_context(tc.tile_pool(name="wp", bufs=12))
        qkv_pool = ctx.enter_context(tc.tile_pool(name="qkvp", bufs=2))
        vaug_pool = ctx.enter_context(tc.tile_pool(name="vaugp", bufs=2))
        pt_pool = ctx.enter_context(tc.tile_pool(name="ptp", bufs=5))
        out_pool = ctx.enter_context(tc.tile_pool(name="outp", bufs=8))
        small_pool = ctx.enter_context(tc.tile_pool(name="smallp", bufs=8))
        st_pool = ctx.enter_context(tc.tile_pool(name="stp", bufs=2, space="PSUM"))
        ot_pool = ctx.enter_context(tc.tile_pool(name="otp", bufs=3, space="PSUM"))

        # ---- batched input DMA first (before any const setup) so the
        # descriptor issue overlaps the framework preamble. xt quarter
        # 0 is split by chunk-halves so the first K matmuls can start
        # after ~0.75MB lands.
        xt_sb = xt_pool.tile([128, NCH, M], BF16, name="xt_sb")

        def load_xq(eng, g, clo=0, chi=NCH):
            eng.dma_start(
                xt_sb[:, clo:chi, g * 512:(g + 1) * 512],
                xt_d.ap()[clo * 128:chi * 128, g * 512:(g + 1) * 512].rearrange(
                    "(c p) m -> p c m", p=128
                ),
            )

        wts = {}

        def load_w(nm, p):
            wd = {"q": wq_d, "k": wk_d, "v": wv_d}[nm]
            wt = w_pool.tile([128, NCH, 128], BF16, name=f"w{nm}{p}", tag="wt")
            nc.gpsimd.dma_start(
                wt[:], wd.ap()[p].rearrange("(c p) d -> p c d", p=128)
            )
            wts[(nm, p)] = wt

        load_xq(nc.sync, 0, 0, 4)
        load_w("k", 0)
        load_xq(nc.sync, 0, 4, 8)
        load_w("q", 0)
        load_w("v", 0)
        load_xq(nc.sync, 1)
        load_xq(nc.sync, 2)
        load_xq(nc.sync, 3)

        ident = const_pool.tile([128, 128], BF16, name="ident")
        make_identity(nc, ident[:])
        ones16 = const_pool.tile([128, KC, 1], F32, name="ones16")
        nc.gpsimd.memset(ones16[:], 1.0)

        for p in range(1, 4):
            for nm in ("k", "v", "q"):
                load_w(nm, p)

        # ---------------------------------------------------------------
        # Projection work as fine-grained emission items. Chain-split
        # qkv projections keep the tag="ps" PSUM accumulator between
        # their two items, so items of one chain stay adjacent.
        # ---------------------------------------------------------------
        def make_pair_tiles(p):
            qt = qkv_pool.tile([128, M], BF16, name="qt", tag="qt")
            kt = qkv_pool.tile([128, M], BF16, name="kt", tag="kt")
            vt = qkv_pool.tile([128, M], BF16, name="vt", tag="vt")
            vaug = vaug_pool.tile([128, KC, 130], BF16, name="vaug", tag="vaug")
            return qt, kt, vt, vaug

        def qkv_items(p, g, nm, dst):
            """Two items: contraction chunks 0-3, then 4-7 + copy out."""
            psbox = {}

            def half_a():
                wt = wts[(nm, p)]
                ps = st_pool.tile(
                    [128, 512], F32, name="ps_prj", tag="ps", bufs=1
                )
                psbox[0] = ps
                for c in range(4):
                    nc.tensor.matmul(
                        ps[:],
                        lhsT=wt[:, c, :],
                        rhs=xt_sb[:, c, g * 512:(g + 1) * 512],
                        start=(c == 0),
                        stop=False,
                        skip_group_check=True,
                    )

            def half_b():
                wt = wts[(nm, p)]
                ps = psbox[0]
                for c in range(4, NCH):
                    nc.tensor.matmul(
                        ps[:],
                        lhsT=wt[:, c, :],
                        rhs=xt_sb[:, c, g * 512:(g + 1) * 512],
                        start=False,
                        stop=(c == NCH - 1),
                        skip_group_check=True,
                    )
                nc.vector.tensor_copy(dst[:, g * 512:(g + 1) * 512], ps[:])

            return [half_a, half_b]

        def tr_items(g, vt, vaug):
            """One shared [128,4,128] PSUM frame per block: transpose
            k4=0 runs start=True (zeroing the whole 2KB zero-region
            once), k4=1..3 run start=False so each lands in its still
            pending-zero quarter as a clean write. No per-transpose
            WAR ping-pong with DVE, and one batched copy out.
            (XBAR dma_start_transpose was tried: NaNs + runtime crash.)"""
            frbox = {}

            def tr_a():
                trf = st_pool.tile(
                    [128, 4, 128], BF16, name="trf", tag="ps", bufs=1
                )
                frbox[0] = trf
                for k4 in range(4):
                    kc = g * 4 + k4
                    nc.tensor.matmul(
                        trf[:, k4, :],
                        lhsT=vt[:, kc * 128:(kc + 1) * 128],
                        rhs=ident[:],
                        is_transpose=True,
                        start=(k4 == 0),
                        stop=(k4 == 3),
                        skip_group_check=True,
                    )

            def tr_b():
                trf = frbox[0]
                nc.vector.tensor_copy(
                    vaug[:, g * 4:(g + 1) * 4, :].rearrange(
                        "p k (h x) -> p k h x", h=2
                    )[:, :, :, 0:64],
                    trf.rearrange("p k (h d) -> p k h d", h=2),
                )

            return [tr_a, tr_b]

        def ones_item(vaug):
            def it():
                for hp in range(2):
                    nc.vector.tensor_copy(
                        vaug[:, :, hp * 65 + 64:hp * 65 + 65], ones16[:]
                    )
            return it

        def proj_items(p, qt, kt, vt, vaug):
            """Full projection item list for pairs >= 1."""
            items = [ones_item(vaug)]
            for g in range(4):
                items += qkv_items(p, g, "k", kt)
                items += qkv_items(p, g, "v", vt)
                items += tr_items(g, vt, vaug)
                items += qkv_items(p, g, "q", qt)
            return items

        # Drain of a finished [65,512] PV accumulator pair. Deferred one
        # mq (gated on the next mq's first exp output) so it can never
        # race the PV-stop matmul's systolic drain into PSUM.
        _PENDING_DRAIN = [None]

        def emit_drain(ots, p, mq, gate_pt=None):
            msl = slice(mq * 512, (mq + 1) * 512)
            if gate_pt is not None:
                gate = small_pool.tile([1, 8], F32, name="gate", tag="gate")
                nc.vector.tensor_copy(gate[:], gate_pt[0:1, 0, 0:8])
            rbcs = []
            for hp in range(2):
                sumsb = small_pool.tile([1, 512], F32, name="sumsb", tag="sm")
                nc.vector.tensor_copy(sumsb[:], ots[hp][64:65, :])
                recipb = small_pool.tile([1, 512], F32, name="recipb", tag="sm")
                scratch = small_pool.tile([1, 512], F32, name="scr", tag="sm")
                nc.vector.reciprocal_approx_accurate(
                    recipb[:], sumsb[:], scratch[:]
                )
                rbc = out_pool.tile([64, 512], F32, name="rbc", tag="o64")
                nc.gpsimd.partition_broadcast(rbc[:], recipb[:])
                rbcs.append(rbc)
            for hp in range(2):
                h = 2 * p + hp
                stage = out_pool.tile([64, 512], F32, name="stage", tag="o64")
                nc.vector.tensor_mul(stage[:], ots[hp][0:64, :], rbcs[hp])
                nc.sync.dma_start(o_d.ap()[h * 64:(h + 1) * 64, msl], stage[:])

        # ---- pair 0 upfront: K block0 + Q block0 (gates the score
        # pipeline); V block0 + transposes follow the first scores.
        tiles = {0: make_pair_tiles(0)}
        qt0, kt0, vt0, vaug0 = tiles[0]
        ones_item(vaug0)()
        for itm in qkv_items(0, 0, "k", kt0) + qkv_items(0, 0, "q", qt0):
            itm()
        vb0_items = qkv_items(0, 0, "v", vt0) + tr_items(0, vt0, vaug0)
        # Urgent ordering constraint: items are popped 2 per step, and
        # emission order IS execution order per engine queue, so an
        # item must be EMITTED before the step that emits its first
        # consumer. tr(g) is consumed by PV(kc=4g) at step 4g; with the
        # interleaved order below tr(g) lands at slot 4-3g+... =
        # {g1: slot 2, g2: 5, g3: 8} < {4, 8, 12}. K(g) is consumed by
        # scores(4g) emitted at step 4g-3: K lands at {0, 3, 6}. Q(g)
        # is consumed by scores(16g) emitted at step 16g-3: Q lands at
        # {9, 10, 11} < 13.
        urgent = []
        for g in range(1, 4):
            urgent += qkv_items(0, g, "k", kt0)
            urgent += qkv_items(0, g, "v", vt0)
            urgent += tr_items(g, vt0, vaug0)
        for g in range(1, 4):
            urgent += qkv_items(0, g, "q", qt0)

        # ---- flat score/exp pipeline over all 256 (pair, mq, kc) steps
        pts = {}

        def score_step(j):
            p, r = divmod(j, MQ * KC)
            mq, kc = divmod(r, KC)
            qt, kt = tiles[p][0], tiles[p][1]
            msl = slice(mq * 512, (mq + 1) * 512)
            ksl = slice(kc * 128, (kc + 1) * 128)
            st = st_pool.tile([128, 2, 512], F32, name="st", tag="st")
            for hp in range(2):
                hsl = slice(64 * hp, 64 * (hp + 1))
                nc.tensor.matmul(
                    st[:, hp, :],
                    lhsT=kt[hsl, ksl],
                    rhs=qt[hsl, msl],
                    start=True,
                    stop=True,
                )
            pt = pt_pool.tile([128, 2, 512], BF16, name="pt", tag="pt")
            nc.scalar.activation(pt[:], st[:], EXPF, scale=SCALE)
            pts[j] = pt

        for j in range(LOOKAHEAD):
            score_step(j)
        for itm in vb0_items:
            itm()

        pending, slots = [], {}
        ots = None
        for i in range(STEPS):
            p, r = divmod(i, MQ * KC)
            mq, kc = divmod(r, KC)
            if r == 0 and p + 1 < NPAIR:
                # allocate next pair's tiles and spread its projection
                # items over this pair's steps (finishing ~4 steps
                # early so the cross-pair score lookahead has data).
                tiles[p + 1] = make_pair_tiles(p + 1)
                pending = proj_items(p + 1, *tiles[p + 1])
                first = (len(urgent) + 1) // 2 if urgent else 0
                span = MQ * KC - first - 5
                slots = {
                    i + first + int(round((t + 1) * span / len(pending))): t
                    for t in range(len(pending))
                }
            if kc == 0:
                ot0 = ot_pool.tile([65, 512], F32, name="ot0", tag="ot")
                ot1 = ot_pool.tile([65, 512], F32, name="ot1", tag="ot")
                ots = (ot0, ot1)
            pt = pts.pop(i)
            if kc == 0 and _PENDING_DRAIN[0] is not None:
                emit_drain(*_PENDING_DRAIN[0], gate_pt=pt)
                _PENDING_DRAIN[0] = None
            vaug = tiles[p][3]
            for hp in range(2):
                nc.tensor.matmul(
                    ots[hp][:],
                    lhsT=vaug[:, kc, hp * 65:hp * 65 + 65],
                    rhs=pt[:, hp, :],
                    start=(kc == 0),
                    stop=(kc == KC - 1),
                    skip_group_check=True,
                )
            if i + LOOKAHEAD < STEPS:
                score_step(i + LOOKAHEAD)
            # 2 urgent (pair-0 tail) items per step, then spread items
            if urgent:
                urgent.pop(0)()
            if urgent:
                urgent.pop(0)()
            if i in slots:
                pending[slots[i]]()
            if kc == KC - 1:
                _PENDING_DRAIN[0] = (ots, p, mq)
        # flush the last mq's drain (no later pt to gate on)
        if _PENDING_DRAIN[0] is not None:
            emit_drain(*_PENDING_DRAIN[0], gate_pt=None)
            _PENDING_DRAIN[0] = None
    nc.compile()
    return nc


_NC_CACHE = None


def _get_nc():
    global _NC_CACHE
    if _NC_CACHE is None:
        _NC_CACHE = build_nc()
    return _NC_CACHE


def make_in_maps(x, W_Q, W_K, W_V):
    import ml_dtypes

    BF = ml_dtypes.bfloat16
    x = np.asarray(x, dtype=np.float32)
    W_Q = np.asarray(W_Q, dtype=np.float32)
    W_K = np.asarray(W_K, dtype=np.float32)
    W_V = np.asarray(W_V, dtype=np.float32)

    def prep_w(W, g):
        blk = W[8 * g:8 * g + 8]  # [8, 1024, 64]
        # pair-major [4, 1024, 128]: col = (head%2)*64 + d
        return np.ascontiguousarray(
            blk.reshape(4, 2, N, D).transpose(0, 2, 1, 3).reshape(4, N, 2 * D)
        ).astype(BF)

    xts = [np.ascontiguousarray(x[b].T).astype(BF) for b in range(B)]
    ws = [
        (prep_w(W_Q, g), prep_w(W_K, g), prep_w(W_V, g)) for g in range(2)
    ]
    in_maps = []
    for c in range(NCORES):
        b, g = divmod(c, 2)
        in_maps.append(
            {
                "xt": xts[b],
                "wq": ws[g][0],
                "wk": ws[g][1],
                "wv": ws[g][2],
            }
        )
    return in_maps


def gather_out(results):
    out = np.empty((B, M, N), dtype=np.float32)
    for c in range(NCORES):
        b, g = divmod(c, 2)
        out[b, :, 512 * g:512 * (g + 1)] = results[c]["ot"].T
    return out


def run(x, W_Q, W_K, W_V, **spmd_kwargs):
    nc = _get_nc()
    in_maps = make_in_maps(x, W_Q, W_K, W_V)
    res = bass_utils.run_bass_kernel_spmd(
        nc, in_maps, core_ids=list(range(NCORES)), **spmd_kwargs
    )
    return gather_out(res.results), res


def kernel(x, W_Q, W_K, W_V):
    out, _ = run(x, W_Q, W_K, W_V)
    return out
